# revision 1
# baseline (speedup 1.0000x reference)
"""Multi-head attention (B=2, S=2048, D=1024, H=16) on 8 trn2 NeuronCores.

Sharding: batch x head-group tensor parallel. Core c handles batch b=c//4 and
head group g=c%4 (4 heads = 256 features). Wq/Wk/Wv are split column-wise by
head (rows of the torch-layout weight), Wo row-wise; each core produces a
partial output for its batch which the host sums (row-parallel linear) and
adds bo.

Per-core dataflow (all matmuls f32r on the PE, f32 accumulation in PSUM):
  - host pre-transposes activations (x^T, d-major) and weight shards
  - Q^T,K^T = W^T.T @ x^T        [e on partitions]  (e-contraction for scores)
  - V       = x^T.T @ Wv^T       [s on partitions]  (+bias, +ones column)
  - S^T     = K^T_h.T @ Q^T_h    [k on partitions, K=64 contraction]
  - expS    = exp(0.125*S^T)     (ScalarE, straight from PSUM)
  - ctxU^T/den = V_aug.T @ expS  (ones column of V_aug produces den row)
  - ctx^T   = ctxU^T * bcast(1/den)   (PE ones-outer-product broadcast)
  - y      += ctx^T.T @ Wo^T     (accumulate 4 heads in PSUM)
"""

from contextlib import ExitStack

import numpy as np

import concourse.bass as bass
import concourse.tile as tile
from concourse import bacc, mybir

B, S, D, NH = 2, 2048, 1024, 16
NCORES = 8
GH = 4            # heads per core
DK = D // NH      # 64
E = GH * DK       # 256 local features per core
F32 = mybir.dt.float32
MM_DT = mybir.dt.float32r   # PE matmul dtype (f32r: full-rate reduced-precision)

QC = 512          # q-chunk (free dim of score tiles)
NQC = S // QC     # 4
NKB = S // 128    # 16 key blocks
NKD = D // 128    # 8 contraction panels for projections


def build_bass():
    nc = bacc.Bacc("TRN2", target_bir_lowering=False, debug=False,
                   num_devices=NCORES)

    xqT = nc.declare_dram_parameter("xqT", [D, S], MM_DT, isOutput=False)
    xkT = nc.declare_dram_parameter("xkT", [D, S], MM_DT, isOutput=False)
    xvT = nc.declare_dram_parameter("xvT", [D, S], MM_DT, isOutput=False)
    wqT = nc.declare_dram_parameter("wqT", [D, E], MM_DT, isOutput=False)
    wkT = nc.declare_dram_parameter("wkT", [D, E], MM_DT, isOutput=False)
    wvT = nc.declare_dram_parameter("wvT", [D, E], MM_DT, isOutput=False)
    bq2 = nc.declare_dram_parameter("bq2", [128, 2], F32, isOutput=False)
    bk2 = nc.declare_dram_parameter("bk2", [128, 2], F32, isOutput=False)
    bvb = nc.declare_dram_parameter("bvb", [128, E], F32, isOutput=False)
    woT = nc.declare_dram_parameter("woT", [DK, GH, D], MM_DT, isOutput=False)
    ones1 = nc.declare_dram_parameter("ones1", [128, DK], MM_DT,
                                      isOutput=False)
    vones = nc.declare_dram_parameter("vones", [128, NKB * GH], MM_DT,
                                      isOutput=False)
    y = nc.declare_dram_parameter("y", [S, D], F32, isOutput=True)

    with ExitStack() as ctx:
        tc = ctx.enter_context(tile.TileContext(nc))
        const = ctx.enter_context(tc.tile_pool(name="const", bufs=1))
        persist = ctx.enter_context(tc.tile_pool(name="persist", bufs=1))
        xt = ctx.enter_context(tc.tile_pool(name="xt", bufs=10))
        exps_p = ctx.enter_context(tc.tile_pool(name="exps", bufs=3))
        small = ctx.enter_context(tc.tile_pool(name="small", bufs=2))
        outp = ctx.enter_context(tc.tile_pool(name="outp", bufs=2))
        ps_proj = ctx.enter_context(
            tc.tile_pool(name="ps_proj", bufs=2, space="PSUM"))
        ps_s = ctx.enter_context(
            tc.tile_pool(name="ps_s", bufs=2, space="PSUM"))
        ps_c = ctx.enter_context(
            tc.tile_pool(name="ps_c", bufs=2, space="PSUM"))
        ps_b = ctx.enter_context(
            tc.tile_pool(name="ps_b", bufs=2, space="PSUM"))

        # ---- constants / weights ----
        wq_sb = const.tile([128, NKD, E], MM_DT, tag="wq")
        wk_sb = const.tile([128, NKD, E], MM_DT, tag="wk")
        wv_sb = const.tile([128, NKD, E], MM_DT, tag="wv")
        nc.sync.dma_start(wq_sb[:], wqT[:].rearrange("(k p) e -> p k e", p=128))
        nc.sync.dma_start(wk_sb[:], wkT[:].rearrange("(k p) e -> p k e", p=128))
        nc.sync.dma_start(wv_sb[:], wvT[:].rearrange("(k p) e -> p k e", p=128))
        wo_sb = const.tile([DK, GH, D], MM_DT, tag="wo")
        nc.sync.dma_start(wo_sb[:], woT[:])
        bias_q = const.tile([128, 2], F32, tag="bq")
        bias_k = const.tile([128, 2], F32, tag="bk")
        bv_bc = const.tile([128, E], F32, tag="bv")
        nc.sync.dma_start(bias_q[:], bq2[:])
        nc.sync.dma_start(bias_k[:], bk2[:])
        nc.sync.dma_start(bv_bc[:], bvb[:])
        ones_col = const.tile([128, DK], MM_DT, tag="ones")
        nc.sync.dma_start(ones_col[:], ones1[:])

        QT_sb = persist.tile([128, 2, S], MM_DT, tag="qt")
        KT_sb = persist.tile([128, 2, S], MM_DT, tag="kt")
        V_aug = persist.tile([128, NKB, GH, DK + 1], MM_DT, tag="va")
        nc.sync.dma_start(V_aug[:, :, :, DK:DK + 1], vones[:])
        ctxT = persist.tile([DK, GH, S], MM_DT, tag="ctx")

        # ---- Q^T / K^T projections (x^T streamed in S-halves) ----
        SH = S // 2
        for src, wsb, bias, dst in ((xqT, wq_sb, bias_q, QT_sb),
                                    (xkT, wk_sb, bias_k, KT_sb)):
            for half in range(2):
                panels = []
                for kd in range(NKD):
                    p = xt.tile([128, SH], MM_DT, tag="xt")
                    nc.sync.dma_start(
                        p[:], src[kd * 128:(kd + 1) * 128,
                                  half * SH:(half + 1) * SH])
                    panels.append(p)
                for t in range(2):
                    for qc in range(half * 2, half * 2 + 2):
                        ps = ps_proj.tile([128, QC], F32, tag="psp")
                        lo = qc * QC - half * SH
                        for kd in range(NKD):
                            nc.tensor.matmul(
                                ps[:],
                                wsb[:, kd, t * 128:(t + 1) * 128],
                                panels[kd][:, lo:lo + QC],
                                start=(kd == 0), stop=(kd == NKD - 1))
                        nc.vector.tensor_scalar_add(
                            dst[:, t, qc * QC:(qc + 1) * QC], ps[:],
                            bias[:, t:t + 1])

        # ---- V projection (natural layout, +bias, +ones col) ----
        for half in range(2):
            panels_v = []
            for kd in range(NKD):
                p = xt.tile([128, SH], MM_DT, tag="xt")
                nc.sync.dma_start(
                    p[:], xvT[kd * 128:(kd + 1) * 128,
                              half * SH:(half + 1) * SH])
                panels_v.append(p)
            for st in range(half * 8, half * 8 + 8):
                lo = st * 128 - half * SH
                ps = ps_proj.tile([128, QC], F32, tag="psp")
                for kd in range(NKD):
                    nc.tensor.matmul(
                        ps[:, 0:E],
                        panels_v[kd][:, lo:lo + 128],
                        wv_sb[:, kd, :],
                        start=(kd == 0), stop=(kd == NKD - 1))
                nc.vector.tensor_tensor(
                    out=V_aug[:, st, :, 0:DK], in0=ps[:, 0:E], in1=bv_bc[:],
                    op=mybir.AluOpType.add)

        # ---- attention + normalize ----
        for qc in range(NQC):
            qsl = slice(qc * QC, (qc + 1) * QC)
            for h in range(GH):
                t, hp = divmod(h, 2)
                esl = slice(hp * 64, (hp + 1) * 64)
                psc = ps_c.tile([DK + 1, QC], F32, tag="psc")
                for kb in range(NKB):
                    pss = ps_s.tile([128, QC], F32, tag="pss")
                    nc.tensor.matmul(
                        pss[:],
                        KT_sb[esl, t, kb * 128:(kb + 1) * 128],
                        QT_sb[esl, t, qsl])
                    es = exps_p.tile([128, QC], MM_DT, tag="es")
                    nc.scalar.activation(
                        es[:], pss[:], mybir.ActivationFunctionType.Exp,
                        scale=float(1.0 / np.sqrt(DK)))
                    nc.tensor.matmul(
                        psc[:], V_aug[:, kb, h, :],
                        es[:],
                        start=(kb == 0), stop=(kb == NKB - 1))
                # 1/den as exp(-ln(den)), kept at partition 64 throughout
                # (engines cannot move data across partitions).
                lnd = small.tile([128, QC], F32, tag="lnd")
                nc.scalar.activation(lnd[DK:DK + 1, :], psc[DK:DK + 1, :],
                                     mybir.ActivationFunctionType.Ln)
                rdr = small.tile([128, QC], MM_DT, tag="rdr")
                nc.scalar.activation(rdr[DK:DK + 1, :], lnd[DK:DK + 1, :],
                                     mybir.ActivationFunctionType.Exp,
                                     scale=-1.0)
                psb = ps_b.tile([DK, QC], F32, tag="psb")
                nc.tensor.matmul(psb[:], ones_col[DK:DK + 1, :],
                                 rdr[DK:DK + 1, :])
                cu = small.tile([DK, QC], F32, tag="cu")
                nc.vector.tensor_copy(cu[:], psc[0:DK, :])
                nc.vector.tensor_tensor(
                    out=ctxT[:, h, qsl], in0=cu[:], in1=psb[:],
                    op=mybir.AluOpType.mult)

            # ---- output projection for this q-chunk ----
            for sti in range(QC // 128):
                st = qc * (QC // 128) + sti
                ssl = slice(st * 128, (st + 1) * 128)
                ob = outp.tile([128, D], F32, tag="ob")
                for oc in range(2):
                    pso = ps_proj.tile([128, QC], F32, tag="psp")
                    for h in range(GH):
                        nc.tensor.matmul(
                            pso[:],
                            ctxT[:, h, ssl],
                            wo_sb[:, h, oc * 512:(oc + 1) * 512],
                            start=(h == 0), stop=(h == GH - 1))
                    nc.vector.tensor_copy(ob[:, oc * 512:(oc + 1) * 512], pso[:])
                nc.sync.dma_start(y[ssl, :], ob[:])

    nc.compile()
    return nc


def make_in_maps(query, key, value, Wq, bq, Wk, bk, Wv, bv, Wo, bo):
    query = np.asarray(query, np.float32)
    key = np.asarray(key, np.float32)
    value = np.asarray(value, np.float32)
    Wq, Wk, Wv, Wo = (np.asarray(w, np.float32) for w in (Wq, Wk, Wv, Wo))
    bq, bk, bv = (np.asarray(b_, np.float32) for b_ in (bq, bk, bv))
    in_maps = []
    xT = {}
    for b in range(B):
        xT[b] = (np.ascontiguousarray(query[b].T),
                 np.ascontiguousarray(key[b].T),
                 np.ascontiguousarray(value[b].T))
    for c in range(NCORES):
        b, g = divmod(c, GH)
        sl = slice(g * E, (g + 1) * E)
        qT, kT, vT = xT[b]
        in_maps.append({
            "xqT": qT, "xkT": kT, "xvT": vT,
            "wqT": np.ascontiguousarray(Wq[sl, :].T),
            "wkT": np.ascontiguousarray(Wk[sl, :].T),
            "wvT": np.ascontiguousarray(Wv[sl, :].T),
            "bq2": np.ascontiguousarray(bq[sl].reshape(2, 128).T),
            "bk2": np.ascontiguousarray(bk[sl].reshape(2, 128).T),
            "bvb": np.ascontiguousarray(np.tile(bv[sl][None, :], (128, 1))),
            "woT": np.ascontiguousarray(
                Wo[:, sl].T.reshape(GH, DK, D).transpose(1, 0, 2)),
            "ones1": np.ones((128, DK), np.float32),
            "vones": np.ones((128, NKB * GH), np.float32),
        })
    return in_maps


_NC_CACHE = {}


def _get_nc():
    if "nc" not in _NC_CACHE:
        _NC_CACHE["nc"] = build_bass()
    return _NC_CACHE["nc"]


def kernel(query, key, value, Wq, bq, Wk, bk, Wv, bv, Wo, bo, **_):
    from concourse import bass_utils

    nc = _get_nc()
    in_maps = make_in_maps(query, key, value, Wq, bq, Wk, bk, Wv, bv, Wo, bo)
    res = bass_utils.run_bass_kernel_spmd(nc, in_maps, list(range(NCORES)))
    parts = [np.asarray(r["y"], np.float32) for r in res.results]
    bo = np.asarray(bo, np.float32)
    out = np.empty((B, S, D), np.float32)
    for b in range(B):
        out[b] = parts[4 * b] + parts[4 * b + 1] + parts[4 * b + 2] \
            + parts[4 * b + 3] + bo
    return out



# revision 2
# speedup vs baseline: 18.9560x; 18.9560x over previous
"""Multi-head attention (B=2, S=2048, D=1024, H=16) on 8 trn2 NeuronCores.

Sharding: batch x head-group tensor parallel. Core c handles batch b=c//4
and head group g=c%4 (4 heads = 256 features). Wq/Wk/Wv split column-wise
by head, Wo row-wise; each core produces a partial output for its batch
which the host sums (row-parallel linear) and adds bo.

Dataflow (bf16 matmul operands, f32 PSUM accumulation), software-pipelined
so the ScalarE exp stream (the irreducible ~110us/core of softmax work)
never starves and the PE tensor engine stays fed:
  - lead-in: K(t0)/Q(t0, first q-half) projected as soon as their d-major
    bf16 panels land; attention starts ~15-20us in
  - per phase (q-512 window, head pair): 16 key blocks of
    score (both heads packed into disjoint PE row groups via base
    partitions 0/64 -> tile_position row packing on HW), 1024-wide exp
    (ScalarE, from 2 PSUM banks), PV per head (128-deep contraction,
    V_aug padded to 128 cols [V|ones|0...] so FWL stays enabled; the ones
    column emits the softmax denominator at PSUM partition 64)
  - all remaining projection / output-projection work is split into
    ~0.85us matmul chunks drained one-per-key-block between exp and PV
    (delaying only PV, which intentionally trails the exp stream)
  - normalize: 1/den via DVE reciprocal; ctx staged to SBUF; the den
    broadcast (PE ones outer product) + multiplies are deferred into the
    next phase so they never block the next score->exp stream; odd heads
    are partition-shifted 0:64 -> 64:128 by a small SBUF-to-SBUF DMA to
    enable the 128-deep output-projection contraction
  - y = ctx2.T @ Wo2 per 128-row s-tile, staged via ScalarE/DVE copies,
    DMA'd out as columns complete
"""

from contextlib import ExitStack

import numpy as np

import concourse.bass as bass
import concourse.tile as tile
from concourse import bacc, mybir

B, S, D, NH = 2, 2048, 1024, 16
NCORES = 8
GH = 4            # heads per core
DK = D // NH      # 64
E = GH * DK       # 256 local features per core
F32 = mybir.dt.float32
F32R = mybir.dt.float32r
BF16 = mybir.dt.bfloat16

QH = 1024         # attention q-chunk (PSUM tile free dim, 2 banks)
NQH = S // QH     # 2
NKB = S // 128    # 16 key blocks
NKD = D // 128    # 8 contraction panels for projections


def build_bass(reps=1):
    nc = bacc.Bacc("TRN2", target_bir_lowering=False, debug=False,
                   num_devices=NCORES)

    xqT = nc.declare_dram_parameter("xqT", [D, S], BF16, isOutput=False)
    xkT = nc.declare_dram_parameter("xkT", [D, S], BF16, isOutput=False)
    xvT = nc.declare_dram_parameter("xvT", [D, S], BF16, isOutput=False)
    wqT = nc.declare_dram_parameter("wqT", [D, E], BF16, isOutput=False)
    wkT = nc.declare_dram_parameter("wkT", [D, E], BF16, isOutput=False)
    wvT = nc.declare_dram_parameter("wvT", [D, E], BF16, isOutput=False)
    bq2 = nc.declare_dram_parameter("bq2", [128, 2], F32, isOutput=False)
    bk2 = nc.declare_dram_parameter("bk2", [128, 2], F32, isOutput=False)
    bvb = nc.declare_dram_parameter("bvb", [128, 2, QH], BF16,
                                    isOutput=False)
    wo2 = nc.declare_dram_parameter("wo2", [128, 2, D], BF16, isOutput=False)
    ones1 = nc.declare_dram_parameter("ones1", [128, DK], F32R,
                                      isOutput=False)
    vones = nc.declare_dram_parameter("vones", [128, 2 * NKB * GH], BF16,
                                      isOutput=False)
    y = nc.declare_dram_parameter("y", [S, D], F32, isOutput=True)

    with ExitStack() as ctx:
        tc = ctx.enter_context(tile.TileContext(nc))
        const = ctx.enter_context(tc.tile_pool(name="const", bufs=1))
        persist = ctx.enter_context(tc.tile_pool(name="persist", bufs=1))
        xt = ctx.enter_context(tc.tile_pool(name="xt", bufs=16))
        xtv = ctx.enter_context(tc.tile_pool(name="xtv", bufs=16))
        xtq = ctx.enter_context(tc.tile_pool(name="xtq", bufs=16))
        es_p = ctx.enter_context(tc.tile_pool(name="es", bufs=8))
        rdr_p = ctx.enter_context(tc.tile_pool(name="rdr", bufs=3))
        ctx_p = ctx.enter_context(tc.tile_pool(name="ctx2", bufs=2))
        outp = ctx.enter_context(tc.tile_pool(name="outp", bufs=2))
        ps_a = ctx.enter_context(
            tc.tile_pool(name="ps_a", bufs=2, space="PSUM"))
        ps_w = ctx.enter_context(
            tc.tile_pool(name="ps_w", bufs=1, space="PSUM"))
        ps_c = ctx.enter_context(
            tc.tile_pool(name="ps_c", bufs=1, space="PSUM"))

        # ---- constants / weights (issued in consumption order) ----
        wq_sb = const.tile([128, NKD, E], BF16, tag="wq")
        wk_sb = const.tile([128, NKD, E], BF16, tag="wk")
        wv_sb = const.tile([128, NKD, E], BF16, tag="wv")
        wo_sb = const.tile([128, 2, D], BF16, tag="wo")
        bias_q = const.tile([128, 2], F32, tag="bq")
        bias_k = const.tile([128, 2], F32, tag="bk")
        bv_bc = const.tile([128, 2, QH], BF16, tag="bv")
        ones_col = const.tile([128, DK], F32R, tag="ones")

        QT_sb = persist.tile([128, 2, S], BF16, tag="qt")
        KT_sb = persist.tile([128, 2, S], BF16, tag="kt")
        # V_aug cols: [V(64) | ones | zeros(63)] -> den at PV out
        # partition 64; 128-wide stationary keeps FWL enabled on HW.
        V_aug = persist.tile([128, NKB, GH, 128], BF16, tag="va")

        for rep in range(reps):
            _body(nc, rep, locals())
    nc.compile()
    return nc


def _body(nc, rep, env):
    (ctx, tc, const, persist, xt, xtv, xtq, es_p, rdr_p, ctx_p, outp,
     ps_a, ps_w, ps_c) = (env["ctx"], env["tc"], env["const"],
                          env["persist"], env["xt"], env["xtv"],
                          env["xtq"], env["es_p"], env["rdr_p"],
                          env["ctx_p"], env["outp"], env["ps_a"],
                          env["ps_w"], env["ps_c"])
    (xqT, xkT, xvT, bvb, wo2, ones1, vones, y) = (
        env["xqT"], env["xkT"], env["xvT"], env["bvb"], env["wo2"],
        env["ones1"], env["vones"], env["y"])
    (wq_sb, wk_sb, wv_sb, wo_sb, bias_q, bias_k, bv_bc, ones_col,
     QT_sb, KT_sb, V_aug) = (
        env["wq_sb"], env["wk_sb"], env["wv_sb"], env["wo_sb"],
        env["bias_q"], env["bias_k"], env["bv_bc"], env["ones_col"],
        env["QT_sb"], env["KT_sb"], env["V_aug"])
    wqT, wkT, wvT, bq2, bk2 = (env["wqT"], env["wkT"], env["wvT"],
                               env["bq2"], env["bk2"])
    if True:
        # ---- projections (software-pipelined with attention) ----
        # Lead-in: xk DMA -> K proj (both pairs), xv -> V(t0), xq ->
        # Q(t0, qh0); then attention starts. Remaining projection and
        # output-projection work is injected into the PE bubbles of the
        # ScalarE-bound attention phases.
        def load_panel_cols(src, pool, qh, panels, width=QH):
            # panels for one qh column half, all 8 kd slices.
            for w0 in range(qh * QH, (qh + 1) * QH, width):
                for kd in range(NKD):
                    p = pool.tile([128, width], BF16, tag="xt",
                                  name=f"pan_{src.name}_{kd}_{w0}_{rep}")
                    nc.sync.dma_start(
                        p[:], src[kd * 128:(kd + 1) * 128, w0:w0 + width])
                    panels[kd][w0 // width] = p

        def proj_ekq_unit(panels, wsb, bias, dst, t, qh, pool=None):
            for c in ekq_chunks(panels, wsb, bias, dst, t, qh, pool):
                c()

        def ekq_chunks(panels, wsb, bias, dst, t, qh, pool=None,
                       hqs=(0, 1)):
            # e-major projection split into ~0.85us matmul chunks so it
            # can drain one-per-kb inside attention without starving
            # the ScalarE exp stream. Panels may be QH- or 512-wide.
            pool = pool or ps_w
            st8 = {}

            def chunk(hq, k0):
                if "ps" not in st8:
                    st8["ps"] = pool.tile(
                        [128, QH], F32,
                        tag="ssa" if pool is ps_a else "psw",
                        name=f"pp_{dst.name}_{t}_{qh}_{rep}")
                ps = st8["ps"]
                for kd in range(k0, k0 + 4):
                    pan = panels[kd][qh]
                    if pan.shape[-1] == QH:
                        mv = pan[:, hq * 512:(hq + 1) * 512]
                    else:
                        mv = panels[kd][qh * 2 + hq][:, :]
                    nc.tensor.matmul(
                        ps[:, hq * 512:(hq + 1) * 512],
                        wsb[:, kd, t * 128:(t + 1) * 128],
                        mv,
                        start=(kd == 0), stop=(kd == NKD - 1))
                if k0 == 4:
                    # per-hq bias add so each 512 q-window of dst
                    # completes as soon as its chunks are done
                    q0 = qh * QH + hq * 512
                    nc.vector.tensor_scalar_add(
                        dst[:, t, q0:q0 + 512],
                        ps[:, hq * 512:(hq + 1) * 512], bias[:, t:t + 1])

            return [lambda a=hq, b=k0: chunk(a, b)
                    for hq in hqs for k0 in (0, 4)]

        def proj_v_unit(vpan, t, half, pool=None):
            for c in v_chunks(vpan, t, half, pool):
                c()

        def v_chunks(vpan, t, half, pool=None):
            # V projection (s-major) in 2-s-tile chunks.
            pool = pool or ps_w
            st8 = {}

            def chunk(s0, last):
                if "ps" not in st8:
                    st8["ps"] = pool.tile(
                        [128, QH], F32,
                        tag="ssa" if pool is ps_a else "psw",
                        name=f"pv_{t}_{half}_{rep}")
                ps = st8["ps"]
                for stl in range(s0, s0 + 2):
                    for kd in range(NKD):
                        nc.tensor.matmul(
                            ps[:, stl * 128:(stl + 1) * 128],
                            vpan[kd][half][:, stl * 128:(stl + 1) * 128],
                            wv_sb[:, kd, t * 128:(t + 1) * 128],
                            start=(kd == 0), stop=(kd == NKD - 1))
                if last:
                    nc.vector.tensor_tensor(
                        out=V_aug[:, half * 8:half * 8 + 8,
                                  2 * t:2 * t + 2, 0:DK],
                        in0=ps[:], in1=bv_bc[:, t, :],
                        op=mybir.AluOpType.add)

            return [lambda a=s0: chunk(a, a == 6) for s0 in (0, 2, 4, 6)]

        ctx2s = [None, None]

        def outproj_unit(qh, st, copy_eng, pool=None):
            s0 = qh * QH + st * 128
            pool = pool or ps_w
            pso = pool.tile(
                [128, QH], F32, tag="ssa" if pool is ps_a else "psw",
                name=f"pso_{qh}_{st}_{rep}")
            ob = outp.tile([128, D], F32, tag="ob")
            for oc in range(2):
                for t in range(2):
                    nc.tensor.matmul(
                        pso[:, oc * 512:(oc + 1) * 512],
                        ctx2s[qh][:, t, st * 128:(st + 1) * 128],
                        wo_sb[:, t, oc * 512:(oc + 1) * 512],
                        start=(t == 0), stop=(t == 1))
                osl = slice(oc * 512, (oc + 1) * 512)
                if copy_eng == "act":
                    nc.scalar.copy(ob[:, osl], pso[:, osl])
                else:
                    nc.vector.tensor_copy(ob[:, osl], pso[:, osl])
                nc.sync.dma_start(y[s0:s0 + 128, osl], ob[:, osl])

        # ---- DMA issue order: feed phase 1 (t0, q 0:512) first ----
        kpan = [[None] * NQH for _ in range(NKD)]
        qpan = [[None] * (2 * NQH) for _ in range(NKD)]
        vpan = [[None] * NQH for _ in range(NKD)]
        if rep == 0:
            nc.sync.dma_start(wk_sb[:],
                              wkT[:].rearrange("(k p) e -> p k e", p=128))
            nc.sync.dma_start(bias_k[:], bk2[:])
        load_panel_cols(xkT, xt, 0, kpan)
        if rep == 0:
            nc.sync.dma_start(wq_sb[:],
                              wqT[:].rearrange("(k p) e -> p k e", p=128))
            nc.sync.dma_start(bias_q[:], bq2[:])
        load_panel_cols(xqT, xtq, 0, qpan)
        load_panel_cols(xkT, xt, 1, kpan)
        if rep == 0:
            nc.sync.dma_start(wv_sb[:],
                              wvT[:].rearrange("(k p) e -> p k e", p=128))
            nc.sync.dma_start(V_aug[:, :, :, DK:DK + 1],
                              vones[:, 0:NKB * GH])
            nc.vector.memset(V_aug[:, :, :, DK + 1:128], 0.0)
            nc.sync.dma_start(bv_bc[:], bvb[:])
        load_panel_cols(xvT, xtv, 0, vpan)
        load_panel_cols(xvT, xtv, 1, vpan)
        load_panel_cols(xqT, xtq, 1, qpan)
        if rep == 0:
            nc.sync.dma_start(ones_col[:], ones1[:])
            nc.sync.dma_start(wo_sb[:], wo2[:])

        # Phase-gated chunk queue: ~0.85us PE chunks drained one per kb
        # between exp and PV, so injected work delays PV (which trails
        # anyway), never the score->exp stream. min_phase gates chunks
        # whose inputs (DMA'd panels / completed ctx2 columns) are not
        # ready earlier.
        workq = []
        workq += [(0, 2, c) for c in ekq_chunks(kpan, wk_sb, bias_k,
                                                KT_sb, 1, 0)]
        workq += [(0, 2, c) for c in ekq_chunks(kpan, wk_sb, bias_k,
                                                KT_sb, 1, 1)]
        workq += [(0, 2, c) for c in ekq_chunks(qpan, wq_sb, bias_q,
                                                QT_sb, 1, 0)]
        workq += [(0, 2, c) for c in v_chunks(vpan, 1, 0)]
        workq += [(0, 2, c) for c in v_chunks(vpan, 1, 1)]
        workq += [(2, 4, c) for c in ekq_chunks(qpan, wq_sb, bias_q,
                                                QT_sb, 0, 1)]
        workq += [(2, 6, c) for c in ekq_chunks(qpan, wq_sb, bias_q,
                                                QT_sb, 1, 1)]
        # outproj s-tiles become available as their ctx2 column windows
        # complete: qh0 iq0 after phase 2, qh0 iq1 after phase 3, ...
        workq += [(3.3, 8, lambda s=s: outproj_unit(0, s, "dve"))
                  for s in range(4)]
        workq += [(4.3, 8, lambda s=s: outproj_unit(0, s, "dve"))
                  for s in range(4, 8)]
        workq += [(7.3, 9, lambda s=s: outproj_unit(1, s, "dve"))
                  for s in range(4)]
        workq += [(8, 9, lambda s=s: outproj_unit(1, s,
                                                  "act" if s % 2 else "dve",
                                                  pool=ps_a if s % 2
                                                  else None))
                  for s in range(4, 8)]

        def normalize_tail(qh, t, iq, cu, rdr):
            # bcast 1/den + multiplies; runs at kb2 of the NEXT phase so
            # the bcast matmul never blocks the next score->exp stream.
            qsl = slice(iq * 512, (iq + 1) * 512)
            psb = ps_a.tile([128, QH], F32, tag="ssa",
                            name=f"psb_{qh}_{t}_{iq}_{rep}")
            for hp in range(2):
                bsl = slice(hp * 512, (hp + 1) * 512)
                nc.tensor.matmul(
                    psb[0:DK, bsl], ones_col[DK:DK + 1, :],
                    rdr[DK:DK + 1, bsl])
            nc.vector.tensor_tensor(
                out=ctx2s[qh][0:64, t, qsl],
                in0=psb[0:64, 0:512], in1=cu[0:64, 0:512],
                op=mybir.AluOpType.mult)
            # odd head: normalize at partitions 0:64, then DMA
            # partition-shift into ctx2[64:128] for the 128-deep
            # output-projection contraction.
            codd = rdr_p.tile([128, 512], BF16, tag="codd")
            nc.vector.tensor_tensor(
                out=codd[0:64, :],
                in0=psb[0:64, 512:1024], in1=cu[0:64, 512:1024],
                op=mybir.AluOpType.mult)
            nc.sync.dma_start(ctx2s[qh][64:128, t, qsl], codd[0:64, :])

        # ---- lead-in projections: just K(t0) + Q(t0, qh0) ----
        proj_ekq_unit(kpan, wk_sb, bias_k, KT_sb, 0, 0)
        proj_ekq_unit(qpan, wq_sb, bias_q, QT_sb, 0, 0, pool=ps_a)
        proj_ekq_unit(kpan, wk_sb, bias_k, KT_sb, 0, 1)

        # ---- attention ----
        # Pair-packed: both heads of pair t per phase. The two score
        # matmuls use stationary base partitions 0 and 64 -> bass derives
        # tile_position (0,0)/(64,0), so they overlap in disjoint PE row
        # groups on HW. PV is full-128-contraction per head into a
        # single-bank [128, 512] accumulator.
        pending = [None]
        for qh in range(NQH):
            ctx2s[qh] = ctx_p.tile([128, 2, QH], BF16, tag="ctx2",
                                   name=f"ctx2_{qh}_{rep}")
            for t in range(2):
                for iq in range(2):
                    first = (qh, t, iq) == (0, 0, 0)
                    pidx = qh * 4 + t * 2 + iq
                    q0 = qh * QH + iq * 512
                    psc = ps_c.tile([128, QH], F32, tag="psc",
                                    name=f"psc_{qh}_{t}_{iq}_{rep}")
                    pv_backlog = []

                    def pv(kb, es):
                        for hp in range(2):
                            nc.tensor.matmul(
                                psc[:, hp * 512:(hp + 1) * 512],
                                V_aug[:, kb, 2 * t + hp, :],
                                es[:, hp * 512:(hp + 1) * 512],
                                start=(kb == 0), stop=(kb == NKB - 1))

                    for kb in range(NKB):
                        ss = ps_a.tile([128, QH], F32, tag="ssa")
                        for hp in range(2):
                            esl = slice(hp * 64, hp * 64 + 64)
                            nc.tensor.matmul(
                                ss[:, hp * 512:(hp + 1) * 512],
                                KT_sb[esl, t, kb * 128:(kb + 1) * 128],
                                QT_sb[esl, t, q0:q0 + 512])
                        es = es_p.tile([128, QH], BF16, tag="es")
                        nc.scalar.activation(
                            es[:], ss[:], mybir.ActivationFunctionType.Exp,
                            scale=float(1.0 / np.sqrt(DK)))
                        if kb == 2 and pending[0] is not None:
                            pending[0]()
                            pending[0] = None
                        if first and kb < 7:
                            # scores/exp run ahead while xv is still in
                            # flight; V(t0) projects here, PV catches up.
                            pv_backlog.append((kb, es))
                            if kb == 6:
                                proj_v_unit(vpan, 0, 0)
                                proj_v_unit(vpan, 0, 1, pool=ps_a)
                                for kb_, es_ in pv_backlog:
                                    pv(kb_, es_)
                            elif (kb >= 1 and workq and workq[0][0]
                                    <= pidx + (0.3 if kb >= 3 else 0)):
                                workq.pop(0)[2]()
                        else:
                            if (kb >= 1 and workq and workq[0][0]
                                    <= pidx + (0.3 if kb >= 3 else 0)):
                                workq.pop(0)[2]()
                            pv(kb, es)

                    rdr = rdr_p.tile([128, QH], F32R, tag="rdr",
                                     name=f"rdr_{rep}")
                    with nc.allow_low_precision(
                            reason="f32r view holds full f32 bits"):
                        nc.vector.reciprocal(rdr[DK:DK + 1, :],
                                             psc[DK:DK + 1, :])
                    if pidx < 7:
                        while workq and workq[0][1] <= pidx + 1:
                            workq.pop(0)[2]()
                    cu = rdr_p.tile([128, QH], F32, tag="cu")
                    nc.vector.tensor_copy(cu[0:64, :], psc[0:64, :])
                    pending[0] = (lambda a=qh, b=t, c=iq, d=cu, e=rdr:
                                  normalize_tail(a, b, c, d, e))

        pending[0]()
        while workq:
            workq.pop(0)[2]()


def make_in_maps(query, key, value, Wq, bq, Wk, bk, Wv, bv, Wo, bo):
    import ml_dtypes
    bf16 = ml_dtypes.bfloat16

    query = np.asarray(query, np.float32)
    key = np.asarray(key, np.float32)
    value = np.asarray(value, np.float32)
    Wq, Wk, Wv, Wo = (np.asarray(w, np.float32) for w in (Wq, Wk, Wv, Wo))
    bq, bk, bv = (np.asarray(b_, np.float32) for b_ in (bq, bk, bv))
    in_maps = []
    xT = {}
    for b in range(B):
        xT[b] = (np.ascontiguousarray(query[b].astype(bf16).T),
                 np.ascontiguousarray(key[b].astype(bf16).T),
                 np.ascontiguousarray(value[b].astype(bf16).T))
    ones1 = np.ones((128, DK), np.float32)
    vones = np.ones((128, 2 * NKB * GH), bf16)
    for c in range(NCORES):
        b, g = divmod(c, GH)
        sl = slice(g * E, (g + 1) * E)
        qT, kT, vT = xT[b]
        bvs = bv[sl]
        bvb = np.stack([np.tile(bvs[t * 128:(t + 1) * 128], QH // 128)
                        for t in range(2)])
        in_maps.append({
            "xqT": qT, "xkT": kT, "xvT": vT,
            "wqT": np.ascontiguousarray(Wq[sl, :].T.astype(bf16)),
            "wkT": np.ascontiguousarray(Wk[sl, :].T.astype(bf16)),
            "wvT": np.ascontiguousarray(Wv[sl, :].T.astype(bf16)),
            "bq2": np.ascontiguousarray(bq[sl].reshape(2, 128).T),
            "bk2": np.ascontiguousarray(bk[sl].reshape(2, 128).T),
            "bvb": np.ascontiguousarray(
                np.broadcast_to(bvb[None], (128, 2, QH)).astype(np.float32)),
            "wo2": np.ascontiguousarray(
                Wo[:, sl].T.reshape(2, 128, D).transpose(1, 0, 2)
                .astype(bf16)),
            "ones1": ones1,
            "vones": vones,
        })
    return in_maps


_NC_CACHE = {}


def _get_nc():
    if "nc" not in _NC_CACHE:
        _NC_CACHE["nc"] = build_bass()
    return _NC_CACHE["nc"]


def kernel(query, key, value, Wq, bq, Wk, bk, Wv, bv, Wo, bo, **_):
    from concourse import bass_utils

    nc = _get_nc()
    in_maps = make_in_maps(query, key, value, Wq, bq, Wk, bk, Wv, bv, Wo, bo)
    res = bass_utils.run_bass_kernel_spmd(nc, in_maps, list(range(NCORES)))
    parts = [np.asarray(r["y"], np.float32) for r in res.results]
    bo = np.asarray(bo, np.float32)
    out = np.empty((B, S, D), np.float32)
    for b in range(B):
        out[b] = parts[4 * b] + parts[4 * b + 1] + parts[4 * b + 2] \
            + parts[4 * b + 3] + bo
    return out


# revision 3
# speedup vs baseline: 19.0700x; 1.0060x over previous
"""Multi-head attention (B=2, S=2048, D=1024, H=16) on 8 trn2 NeuronCores.

Sharding: batch x head-group tensor parallel. Core c handles batch b=c//4
and head group g=c%4 (4 heads = 256 features). Wq/Wk/Wv split column-wise
by head, Wo row-wise; each core produces a partial output for its batch
which the host sums (row-parallel linear) and adds bo.

Dataflow (bf16 matmul operands, f32 PSUM accumulation), software-pipelined
so the ScalarE exp stream (the irreducible ~110us/core of softmax work)
never starves and the PE tensor engine stays fed:
  - lead-in: K(t0)/Q(t0, first q-half) projected as soon as their d-major
    bf16 panels land; attention starts ~15-20us in
  - per phase (q-512 window, head pair): 16 key blocks of
    score (both heads packed into disjoint PE row groups via base
    partitions 0/64 -> tile_position row packing on HW), 1024-wide exp
    (ScalarE, from 2 PSUM banks), PV per head (128-deep contraction,
    V_aug padded to 128 cols [V|ones|0...] so FWL stays enabled; the ones
    column emits the softmax denominator at PSUM partition 64)
  - all remaining projection / output-projection work is split into
    ~0.85us matmul chunks drained one-per-key-block between exp and PV
    (delaying only PV, which intentionally trails the exp stream)
  - normalize: 1/den via DVE reciprocal; ctx staged to SBUF; the den
    broadcast (PE ones outer product) + multiplies are deferred into the
    next phase so they never block the next score->exp stream; odd heads
    are partition-shifted 0:64 -> 64:128 by a small SBUF-to-SBUF DMA to
    enable the 128-deep output-projection contraction
  - y = ctx2.T @ Wo2 per 128-row s-tile, staged via ScalarE/DVE copies
    to bf16, DMA'd out as columns complete; host sums the 4 per-batch
    partials in f32 and adds bo
"""

from contextlib import ExitStack

import numpy as np

import concourse.bass as bass
import concourse.tile as tile
from concourse import bacc, mybir

B, S, D, NH = 2, 2048, 1024, 16
NCORES = 8
GH = 4            # heads per core
DK = D // NH      # 64
E = GH * DK       # 256 local features per core
F32 = mybir.dt.float32
F32R = mybir.dt.float32r
BF16 = mybir.dt.bfloat16

QH = 1024         # attention q-chunk (PSUM tile free dim, 2 banks)
NQH = S // QH     # 2
NKB = S // 128    # 16 key blocks
NKD = D // 128    # 8 contraction panels for projections


def build_bass(reps=1):
    nc = bacc.Bacc("TRN2", target_bir_lowering=False, debug=False,
                   num_devices=NCORES)

    xqT = nc.declare_dram_parameter("xqT", [D, S], BF16, isOutput=False)
    xkT = nc.declare_dram_parameter("xkT", [D, S], BF16, isOutput=False)
    xvT = nc.declare_dram_parameter("xvT", [D, S], BF16, isOutput=False)
    wqT = nc.declare_dram_parameter("wqT", [D, E], BF16, isOutput=False)
    wkT = nc.declare_dram_parameter("wkT", [D, E], BF16, isOutput=False)
    wvT = nc.declare_dram_parameter("wvT", [D, E], BF16, isOutput=False)
    bq2 = nc.declare_dram_parameter("bq2", [128, 2], F32, isOutput=False)
    bk2 = nc.declare_dram_parameter("bk2", [128, 2], F32, isOutput=False)
    bvb = nc.declare_dram_parameter("bvb", [128, 2, QH], BF16,
                                    isOutput=False)
    wo2 = nc.declare_dram_parameter("wo2", [128, 2, D], BF16, isOutput=False)
    ones1 = nc.declare_dram_parameter("ones1", [128, DK], F32R,
                                      isOutput=False)
    vones = nc.declare_dram_parameter("vones", [128, 2 * NKB * GH], BF16,
                                      isOutput=False)
    y = nc.declare_dram_parameter("y", [S, D], BF16, isOutput=True)

    with ExitStack() as ctx:
        tc = ctx.enter_context(tile.TileContext(nc))
        const = ctx.enter_context(tc.tile_pool(name="const", bufs=1))
        persist = ctx.enter_context(tc.tile_pool(name="persist", bufs=1))
        xt = ctx.enter_context(tc.tile_pool(name="xt", bufs=16))
        xtv = ctx.enter_context(tc.tile_pool(name="xtv", bufs=16))
        xtq = ctx.enter_context(tc.tile_pool(name="xtq", bufs=16))
        es_p = ctx.enter_context(tc.tile_pool(name="es", bufs=8))
        rdr_p = ctx.enter_context(tc.tile_pool(name="rdr", bufs=3))
        ctx_p = ctx.enter_context(tc.tile_pool(name="ctx2", bufs=2))
        outp = ctx.enter_context(tc.tile_pool(name="outp", bufs=2))
        ps_a = ctx.enter_context(
            tc.tile_pool(name="ps_a", bufs=2, space="PSUM"))
        ps_w = ctx.enter_context(
            tc.tile_pool(name="ps_w", bufs=1, space="PSUM"))
        ps_c = ctx.enter_context(
            tc.tile_pool(name="ps_c", bufs=1, space="PSUM"))

        # ---- constants / weights (issued in consumption order) ----
        wq_sb = const.tile([128, NKD, E], BF16, tag="wq")
        wk_sb = const.tile([128, NKD, E], BF16, tag="wk")
        wv_sb = const.tile([128, NKD, E], BF16, tag="wv")
        wo_sb = const.tile([128, 2, D], BF16, tag="wo")
        bias_q = const.tile([128, 2], F32, tag="bq")
        bias_k = const.tile([128, 2], F32, tag="bk")
        bv_bc = const.tile([128, 2, QH], BF16, tag="bv")
        ones_col = const.tile([128, DK], F32R, tag="ones")

        QT_sb = persist.tile([128, 2, S], BF16, tag="qt")
        KT_sb = persist.tile([128, 2, S], BF16, tag="kt")
        # V_aug cols: [V(64) | ones | zeros(63)] -> den at PV out
        # partition 64; 128-wide stationary keeps FWL enabled on HW.
        V_aug = persist.tile([128, NKB, GH, 128], BF16, tag="va")

        for rep in range(reps):
            _body(nc, rep, locals())
    nc.compile()
    return nc


def _body(nc, rep, env):
    (ctx, tc, const, persist, xt, xtv, xtq, es_p, rdr_p, ctx_p, outp,
     ps_a, ps_w, ps_c) = (env["ctx"], env["tc"], env["const"],
                          env["persist"], env["xt"], env["xtv"],
                          env["xtq"], env["es_p"], env["rdr_p"],
                          env["ctx_p"], env["outp"], env["ps_a"],
                          env["ps_w"], env["ps_c"])
    (xqT, xkT, xvT, bvb, wo2, ones1, vones, y) = (
        env["xqT"], env["xkT"], env["xvT"], env["bvb"], env["wo2"],
        env["ones1"], env["vones"], env["y"])
    (wq_sb, wk_sb, wv_sb, wo_sb, bias_q, bias_k, bv_bc, ones_col,
     QT_sb, KT_sb, V_aug) = (
        env["wq_sb"], env["wk_sb"], env["wv_sb"], env["wo_sb"],
        env["bias_q"], env["bias_k"], env["bv_bc"], env["ones_col"],
        env["QT_sb"], env["KT_sb"], env["V_aug"])
    wqT, wkT, wvT, bq2, bk2 = (env["wqT"], env["wkT"], env["wvT"],
                               env["bq2"], env["bk2"])
    if True:
        # ---- projections (software-pipelined with attention) ----
        # Lead-in: xk DMA -> K proj (both pairs), xv -> V(t0), xq ->
        # Q(t0, qh0); then attention starts. Remaining projection and
        # output-projection work is injected into the PE bubbles of the
        # ScalarE-bound attention phases.
        def load_panel_cols(src, pool, qh, panels, width=QH):
            # panels for one qh column half, all 8 kd slices.
            for w0 in range(qh * QH, (qh + 1) * QH, width):
                for kd in range(NKD):
                    p = pool.tile([128, width], BF16, tag="xt",
                                  name=f"pan_{src.name}_{kd}_{w0}_{rep}")
                    nc.sync.dma_start(
                        p[:], src[kd * 128:(kd + 1) * 128, w0:w0 + width])
                    panels[kd][w0 // width] = p

        def proj_ekq_unit(panels, wsb, bias, dst, t, qh, pool=None):
            for c in ekq_chunks(panels, wsb, bias, dst, t, qh, pool):
                c()

        def ekq_chunks(panels, wsb, bias, dst, t, qh, pool=None,
                       hqs=(0, 1)):
            # e-major projection split into ~0.85us matmul chunks so it
            # can drain one-per-kb inside attention without starving
            # the ScalarE exp stream. Panels may be QH- or 512-wide.
            pool = pool or ps_w
            st8 = {}

            def chunk(hq, k0):
                if "ps" not in st8:
                    st8["ps"] = pool.tile(
                        [128, QH], F32,
                        tag="ssa" if pool is ps_a else "psw",
                        name=f"pp_{dst.name}_{t}_{qh}_{rep}")
                ps = st8["ps"]
                for kd in range(k0, k0 + 4):
                    pan = panels[kd][qh]
                    if pan.shape[-1] == QH:
                        mv = pan[:, hq * 512:(hq + 1) * 512]
                    else:
                        mv = panels[kd][qh * 2 + hq][:, :]
                    nc.tensor.matmul(
                        ps[:, hq * 512:(hq + 1) * 512],
                        wsb[:, kd, t * 128:(t + 1) * 128],
                        mv,
                        start=(kd == 0), stop=(kd == NKD - 1))
                if k0 == 4:
                    # per-hq bias add so each 512 q-window of dst
                    # completes as soon as its chunks are done
                    q0 = qh * QH + hq * 512
                    nc.vector.tensor_scalar_add(
                        dst[:, t, q0:q0 + 512],
                        ps[:, hq * 512:(hq + 1) * 512], bias[:, t:t + 1])

            return [lambda a=hq, b=k0: chunk(a, b)
                    for hq in hqs for k0 in (0, 4)]

        def proj_v_unit(vpan, t, half, pool=None):
            for c in v_chunks(vpan, t, half, pool):
                c()

        def v_chunks(vpan, t, half, pool=None):
            # V projection (s-major) in 2-s-tile chunks.
            pool = pool or ps_w
            st8 = {}

            def chunk(s0, last):
                if "ps" not in st8:
                    st8["ps"] = pool.tile(
                        [128, QH], F32,
                        tag="ssa" if pool is ps_a else "psw",
                        name=f"pv_{t}_{half}_{rep}")
                ps = st8["ps"]
                for stl in range(s0, s0 + 2):
                    for kd in range(NKD):
                        nc.tensor.matmul(
                            ps[:, stl * 128:(stl + 1) * 128],
                            vpan[kd][half][:, stl * 128:(stl + 1) * 128],
                            wv_sb[:, kd, t * 128:(t + 1) * 128],
                            start=(kd == 0), stop=(kd == NKD - 1))
                if last:
                    nc.vector.tensor_tensor(
                        out=V_aug[:, half * 8:half * 8 + 8,
                                  2 * t:2 * t + 2, 0:DK],
                        in0=ps[:], in1=bv_bc[:, t, :],
                        op=mybir.AluOpType.add)

            return [lambda a=s0: chunk(a, a == 6) for s0 in (0, 2, 4, 6)]

        ctx2s = [None, None]

        def outproj_unit(qh, st, copy_eng, pool=None):
            s0 = qh * QH + st * 128
            pool = pool or ps_w
            pso = pool.tile(
                [128, QH], F32, tag="ssa" if pool is ps_a else "psw",
                name=f"pso_{qh}_{st}_{rep}")
            ob = outp.tile([128, D], BF16, tag="ob")
            for oc in range(2):
                for t in range(2):
                    nc.tensor.matmul(
                        pso[:, oc * 512:(oc + 1) * 512],
                        ctx2s[qh][:, t, st * 128:(st + 1) * 128],
                        wo_sb[:, t, oc * 512:(oc + 1) * 512],
                        start=(t == 0), stop=(t == 1))
                osl = slice(oc * 512, (oc + 1) * 512)
                if copy_eng == "act":
                    nc.scalar.copy(ob[:, osl], pso[:, osl])
                else:
                    nc.vector.tensor_copy(ob[:, osl], pso[:, osl])
                nc.sync.dma_start(y[s0:s0 + 128, osl], ob[:, osl])

        # ---- DMA issue order: feed phase 1 (t0, q 0:512) first ----
        kpan = [[None] * NQH for _ in range(NKD)]
        qpan = [[None] * (2 * NQH) for _ in range(NKD)]
        vpan = [[None] * NQH for _ in range(NKD)]
        if rep == 0:
            nc.sync.dma_start(wk_sb[:],
                              wkT[:].rearrange("(k p) e -> p k e", p=128))
            nc.sync.dma_start(bias_k[:], bk2[:])
        load_panel_cols(xkT, xt, 0, kpan)
        if rep == 0:
            nc.sync.dma_start(wq_sb[:],
                              wqT[:].rearrange("(k p) e -> p k e", p=128))
            nc.sync.dma_start(bias_q[:], bq2[:])
        load_panel_cols(xqT, xtq, 0, qpan)
        load_panel_cols(xkT, xt, 1, kpan)
        if rep == 0:
            nc.sync.dma_start(wv_sb[:],
                              wvT[:].rearrange("(k p) e -> p k e", p=128))
            nc.sync.dma_start(V_aug[:, :, :, DK:DK + 1],
                              vones[:, 0:NKB * GH])
            nc.vector.memset(V_aug[:, :, :, DK + 1:128], 0.0)
            nc.sync.dma_start(bv_bc[:], bvb[:])
        load_panel_cols(xvT, xtv, 0, vpan)
        load_panel_cols(xvT, xtv, 1, vpan)
        load_panel_cols(xqT, xtq, 1, qpan)
        if rep == 0:
            nc.sync.dma_start(ones_col[:], ones1[:])
            nc.sync.dma_start(wo_sb[:], wo2[:])

        # Phase-gated chunk queue: ~0.85us PE chunks drained one per kb
        # between exp and PV, so injected work delays PV (which trails
        # anyway), never the score->exp stream. min_phase gates chunks
        # whose inputs (DMA'd panels / completed ctx2 columns) are not
        # ready earlier.
        workq = []
        workq += [(0, 2, c) for c in ekq_chunks(kpan, wk_sb, bias_k,
                                                KT_sb, 1, 0)]
        workq += [(0, 2, c) for c in ekq_chunks(kpan, wk_sb, bias_k,
                                                KT_sb, 1, 1)]
        workq += [(0, 2, c) for c in ekq_chunks(qpan, wq_sb, bias_q,
                                                QT_sb, 1, 0)]
        workq += [(0, 2, c) for c in v_chunks(vpan, 1, 0)]
        workq += [(0, 2, c) for c in v_chunks(vpan, 1, 1)]
        workq += [(2, 4, c) for c in ekq_chunks(qpan, wq_sb, bias_q,
                                                QT_sb, 0, 1)]
        workq += [(2, 6, c) for c in ekq_chunks(qpan, wq_sb, bias_q,
                                                QT_sb, 1, 1)]
        # outproj s-tiles become available as their ctx2 column windows
        # complete: qh0 iq0 after phase 2, qh0 iq1 after phase 3, ...
        workq += [(3.3, 8, lambda s=s: outproj_unit(0, s, "dve"))
                  for s in range(4)]
        workq += [(4.3, 8, lambda s=s: outproj_unit(0, s, "dve"))
                  for s in range(4, 8)]
        workq += [(7.3, 9, lambda s=s: outproj_unit(1, s, "dve"))
                  for s in range(4)]
        workq += [(8, 9, lambda s=s: outproj_unit(1, s,
                                                  "act" if s % 2 else "dve",
                                                  pool=ps_a if s % 2
                                                  else None))
                  for s in range(4, 8)]

        def normalize_tail(qh, t, iq, cu, rdr):
            # bcast 1/den + multiplies; runs at kb2 of the NEXT phase so
            # the bcast matmul never blocks the next score->exp stream.
            qsl = slice(iq * 512, (iq + 1) * 512)
            psb = ps_a.tile([128, QH], F32, tag="ssa",
                            name=f"psb_{qh}_{t}_{iq}_{rep}")
            for hp in range(2):
                bsl = slice(hp * 512, (hp + 1) * 512)
                nc.tensor.matmul(
                    psb[0:DK, bsl], ones_col[DK:DK + 1, :],
                    rdr[DK:DK + 1, bsl])
            nc.vector.tensor_tensor(
                out=ctx2s[qh][0:64, t, qsl],
                in0=psb[0:64, 0:512], in1=cu[0:64, 0:512],
                op=mybir.AluOpType.mult)
            # odd head: normalize at partitions 0:64, then DMA
            # partition-shift into ctx2[64:128] for the 128-deep
            # output-projection contraction.
            codd = rdr_p.tile([128, 512], BF16, tag="codd")
            nc.vector.tensor_tensor(
                out=codd[0:64, :],
                in0=psb[0:64, 512:1024], in1=cu[0:64, 512:1024],
                op=mybir.AluOpType.mult)
            nc.sync.dma_start(ctx2s[qh][64:128, t, qsl], codd[0:64, :])

        # ---- lead-in projections: just K(t0) + Q(t0, qh0) ----
        proj_ekq_unit(kpan, wk_sb, bias_k, KT_sb, 0, 0)
        proj_ekq_unit(qpan, wq_sb, bias_q, QT_sb, 0, 0, pool=ps_a)
        proj_ekq_unit(kpan, wk_sb, bias_k, KT_sb, 0, 1)

        # ---- attention ----
        # Pair-packed: both heads of pair t per phase. The two score
        # matmuls use stationary base partitions 0 and 64 -> bass derives
        # tile_position (0,0)/(64,0), so they overlap in disjoint PE row
        # groups on HW. PV is full-128-contraction per head into a
        # single-bank [128, 512] accumulator.
        pending = [None]
        for qh in range(NQH):
            ctx2s[qh] = ctx_p.tile([128, 2, QH], BF16, tag="ctx2",
                                   name=f"ctx2_{qh}_{rep}")
            for t in range(2):
                for iq in range(2):
                    first = (qh, t, iq) == (0, 0, 0)
                    pidx = qh * 4 + t * 2 + iq
                    q0 = qh * QH + iq * 512
                    psc = ps_c.tile([128, QH], F32, tag="psc",
                                    name=f"psc_{qh}_{t}_{iq}_{rep}")
                    pv_backlog = []

                    def pv(kb, es):
                        for hp in range(2):
                            nc.tensor.matmul(
                                psc[:, hp * 512:(hp + 1) * 512],
                                V_aug[:, kb, 2 * t + hp, :],
                                es[:, hp * 512:(hp + 1) * 512],
                                start=(kb == 0), stop=(kb == NKB - 1))

                    for kb in range(NKB):
                        ss = ps_a.tile([128, QH], F32, tag="ssa")
                        for hp in range(2):
                            esl = slice(hp * 64, hp * 64 + 64)
                            nc.tensor.matmul(
                                ss[:, hp * 512:(hp + 1) * 512],
                                KT_sb[esl, t, kb * 128:(kb + 1) * 128],
                                QT_sb[esl, t, q0:q0 + 512])
                        es = es_p.tile([128, QH], BF16, tag="es")
                        nc.scalar.activation(
                            es[:], ss[:], mybir.ActivationFunctionType.Exp,
                            scale=float(1.0 / np.sqrt(DK)))
                        if kb == 2 and pending[0] is not None:
                            pending[0]()
                            pending[0] = None
                        if first and kb < 7:
                            # scores/exp run ahead while xv is still in
                            # flight; V(t0) projects here, PV catches up.
                            pv_backlog.append((kb, es))
                            if kb == 6:
                                proj_v_unit(vpan, 0, 0)
                                proj_v_unit(vpan, 0, 1, pool=ps_a)
                                for kb_, es_ in pv_backlog:
                                    pv(kb_, es_)
                            elif (kb >= 1 and workq and workq[0][0]
                                    <= pidx + (0.3 if kb >= 3 else 0)):
                                workq.pop(0)[2]()
                        else:
                            if (kb >= 1 and workq and workq[0][0]
                                    <= pidx + (0.3 if kb >= 3 else 0)):
                                workq.pop(0)[2]()
                            pv(kb, es)

                    rdr = rdr_p.tile([128, QH], F32R, tag="rdr",
                                     name=f"rdr_{rep}")
                    with nc.allow_low_precision(
                            reason="f32r view holds full f32 bits"):
                        nc.vector.reciprocal(rdr[DK:DK + 1, :],
                                             psc[DK:DK + 1, :])
                    if pidx < 7:
                        while workq and workq[0][1] <= pidx + 1:
                            workq.pop(0)[2]()
                    cu = rdr_p.tile([128, QH], F32, tag="cu")
                    nc.vector.tensor_copy(cu[0:64, :], psc[0:64, :])
                    pending[0] = (lambda a=qh, b=t, c=iq, d=cu, e=rdr:
                                  normalize_tail(a, b, c, d, e))

        pending[0]()
        while workq:
            workq.pop(0)[2]()


def make_in_maps(query, key, value, Wq, bq, Wk, bk, Wv, bv, Wo, bo):
    import ml_dtypes
    bf16 = ml_dtypes.bfloat16

    query = np.asarray(query, np.float32)
    key = np.asarray(key, np.float32)
    value = np.asarray(value, np.float32)
    Wq, Wk, Wv, Wo = (np.asarray(w, np.float32) for w in (Wq, Wk, Wv, Wo))
    bq, bk, bv = (np.asarray(b_, np.float32) for b_ in (bq, bk, bv))
    in_maps = []
    xT = {}
    for b in range(B):
        xT[b] = (np.ascontiguousarray(query[b].astype(bf16).T),
                 np.ascontiguousarray(key[b].astype(bf16).T),
                 np.ascontiguousarray(value[b].astype(bf16).T))
    ones1 = np.ones((128, DK), np.float32)
    vones = np.ones((128, 2 * NKB * GH), bf16)
    for c in range(NCORES):
        b, g = divmod(c, GH)
        sl = slice(g * E, (g + 1) * E)
        qT, kT, vT = xT[b]
        bvs = bv[sl]
        bvb = np.stack([np.tile(bvs[t * 128:(t + 1) * 128], QH // 128)
                        for t in range(2)])
        in_maps.append({
            "xqT": qT, "xkT": kT, "xvT": vT,
            "wqT": np.ascontiguousarray(Wq[sl, :].T.astype(bf16)),
            "wkT": np.ascontiguousarray(Wk[sl, :].T.astype(bf16)),
            "wvT": np.ascontiguousarray(Wv[sl, :].T.astype(bf16)),
            "bq2": np.ascontiguousarray(bq[sl].reshape(2, 128).T),
            "bk2": np.ascontiguousarray(bk[sl].reshape(2, 128).T),
            "bvb": np.ascontiguousarray(
                np.broadcast_to(bvb[None], (128, 2, QH)).astype(np.float32)),
            "wo2": np.ascontiguousarray(
                Wo[:, sl].T.reshape(2, 128, D).transpose(1, 0, 2)
                .astype(bf16)),
            "ones1": ones1,
            "vones": vones,
        })
    return in_maps


_NC_CACHE = {}


def _get_nc():
    if "nc" not in _NC_CACHE:
        _NC_CACHE["nc"] = build_bass()
    return _NC_CACHE["nc"]


def kernel(query, key, value, Wq, bq, Wk, bk, Wv, bv, Wo, bo, **_):
    from concourse import bass_utils

    nc = _get_nc()
    in_maps = make_in_maps(query, key, value, Wq, bq, Wk, bk, Wv, bv, Wo, bo)
    res = bass_utils.run_bass_kernel_spmd(nc, in_maps, list(range(NCORES)))
    parts = [np.asarray(r["y"]).astype(np.float32) for r in res.results]
    bo = np.asarray(bo, np.float32)
    out = np.empty((B, S, D), np.float32)
    for b in range(B):
        out[b] = parts[4 * b] + parts[4 * b + 1] + parts[4 * b + 2] \
            + parts[4 * b + 3] + bo
    return out
